# revision 59
# baseline (speedup 1.0000x reference)
# PointNet++ feature-propagation (three_nn + three_interpolate + shared MLP)
# Trainium2 Bass/Tile kernel, 8 NeuronCores, data-parallel over batch.
#
# Per batch (n=4096 unknown, m=1024 known, C2=512, C1=256):
#  1) D[i,j] = 2*u_i.k_j - |k_j|^2 via ONE bf16 matmul with K=21 rows
#     (triple bf16 splits -> ~fp32-accurate v); top-8 per point scanned
#     directly from PSUM (nc.vector.max/max_index), keep top-3
#  2) d2 = max(|u|^2 - v, 0) + EPS straight from the matmul values,
#     inverse-distance weights in fp32
#  3) features: SBUF-source DGE transpose-gather of bf16 feats_t rows
#     (table stays in SBUF), weighted-sum with PE-broadcast weight tiles.
#     Gather descriptors use r = 16*p + t order so the index tile is
#     built with one PE transpose + one replicate-matmul; the permutation
#     is undone in the MLP2 relu copy. Neighbor k=2's weighted features
#     are accumulated by the MLP1 matmul in PSUM (saves a DVE add pass).
#  4) MLP: bf16 matmuls, fp32 PSUM accumulation, relu on ACT, fp32 output
#
# Scheduling: engines run their instruction streams in order, so batch
# b+1's prep/scan is EMITTED inside batch b's gather/MLP phases, and each
# half is processed in two p-split quarters so DVE, PE, ACT and the DMA
# device stay overlapped.
import numpy as np
from contextlib import ExitStack

import concourse.bass as bass
import concourse.bacc as bacc
import concourse.tile as tile
import concourse.mybir as mybir
from concourse.masks import make_identity

AP = bass.AP
dt = mybir.dt
Alu = mybir.AluOpType
ACTF = mybir.ActivationFunctionType

B_FULL = 16
N_CORES = 8
NB = 2            # batches per core
N = 4096
M = 1024
C1 = 256
C2 = 512
D0 = 256
D1 = 256
EPS = 1e-8

NCH = N // 128    # 32 i-chunks
MCH = M // 128    # 8 j-chunks
HALF = N // 2     # 2048
HCH = NCH // 2    # 16 chunks per half
QTR = HALF // 2   # 1024 (processing slice)
KROWS = 21


def _v(t_ap, dims, off=0):
    return AP(t_ap.tensor, t_ap.offset + off, dims)


def build_nc(nb=NB):
    nc = bacc.Bacc("TRN2", target_bir_lowering=False, debug=False)

    unknown_h = nc.dram_tensor("unknown", [nb, N, 3], dt.float32, kind="ExternalInput")
    known_h = nc.dram_tensor("known", [nb, M, 3], dt.float32, kind="ExternalInput")
    uf_h = nc.dram_tensor("unknow_feats", [nb, C1, N], dt.float32, kind="ExternalInput")
    kf_h = nc.dram_tensor("known_feats", [nb, C2, M], dt.float32, kind="ExternalInput")
    w0_h = nc.dram_tensor("W0", [C1 + C2, D0], dt.float32, kind="ExternalInput")
    w1_h = nc.dram_tensor("W1", [D0, D1], dt.float32, kind="ExternalInput")
    out_h = nc.dram_tensor("out", [nb, D1, N], dt.float32, kind="ExternalOutput")

    with tile.TileContext(nc) as tc, ExitStack() as ctx:
        const = ctx.enter_context(tc.tile_pool(name="const", bufs=1))
        init = ctx.enter_context(tc.tile_pool(name="init", bufs=1))
        prep = ctx.enter_context(tc.tile_pool(name="prep", bufs=2))
        sp = ctx.enter_context(tc.tile_pool(name="split", bufs=2))
        sel = ctx.enter_context(tc.tile_pool(name="sel", bufs=2))
        wts = ctx.enter_context(tc.tile_pool(name="wts", bufs=1))
        wtmp = ctx.enter_context(tc.tile_pool(name="wtmp", bufs=2))
        wper = ctx.enter_context(tc.tile_pool(name="wper", bufs=1))
        itp = ctx.enter_context(tc.tile_pool(name="itp", bufs=2))
        gat = ctx.enter_context(tc.tile_pool(name="gat", bufs=2))
        gwp = ctx.enter_context(tc.tile_pool(name="gwp", bufs=2))
        mlpp = ctx.enter_context(tc.tile_pool(name="mlpp", bufs=2))
        ufp = ctx.enter_context(tc.tile_pool(name="ufp", bufs=1))
        outp = ctx.enter_context(tc.tile_pool(name="outp", bufs=2))
        ftp = ctx.enter_context(tc.tile_pool(name="ftp", bufs=1))
        ps_d = ctx.enter_context(tc.tile_pool(name="ps_d", bufs=2, space="PSUM"))
        ps_mm = ctx.enter_context(tc.tile_pool(name="ps_mm", bufs=2, space="PSUM"))
        ps_tr = ctx.enter_context(tc.tile_pool(name="ps_tr", bufs=1, space="PSUM"))
        ps_rp = ctx.enter_context(tc.tile_pool(name="ps_rp", bufs=1, space="PSUM"))

        # ---------------- constants ----------------
        ident_b = const.tile([128, 128], dt.bfloat16, tag="idb")
        make_identity(nc, ident_b[:])
        ident_u = const.tile([128, 128], dt.float16, tag="idu")
        make_identity(nc, ident_u[:])
        ones_b = const.tile([1, 128], dt.bfloat16, tag="ones")
        nc.vector.memset(ones_b[:], 1.0)
        # R16[c, p] = (p % 16 == c): replicates a 16-partition tile to 128
        r16 = const.tile([16, 8, 16], dt.float16, tag="r16")
        for j in range(8):
            nc.scalar.copy(r16[:, j, :], ident_u[:16, :16])

        w0_sb = const.tile([128, 6, D0], dt.bfloat16, tag="w0")
        w1_sb = const.tile([128, 2, D1], dt.bfloat16, tag="w1")
        for ci in range(6):
            w0_f = init.tile([128, D0], dt.float32, tag="w0f")
            nc.sync.dma_start(w0_f[:], w0_h.ap()[128 * ci:128 * ci + 128, :])
            nc.scalar.copy(w0_sb[:, ci, :], w0_f[:])
        for ci in range(2):
            w1_f = init.tile([128, D1], dt.float32, tag="w1f")
            nc.sync.dma_start(w1_f[:], w1_h.ap()[128 * ci:128 * ci + 128, :])
            nc.scalar.copy(w1_sb[:, ci, :], w1_f[:])

        def bf16_split3(x_ap, shape):
            xh = sp.tile(list(shape), dt.bfloat16, tag="sp_h")
            xl = sp.tile(list(shape), dt.bfloat16, tag="sp_l")
            xm = sp.tile(list(shape), dt.bfloat16, tag="sp_m")
            r1 = sp.tile(list(shape), dt.float32, tag="sp_r1")
            r2 = sp.tile(list(shape), dt.float32, tag="sp_r2")
            nc.scalar.copy(xh[:], x_ap)
            nc.vector.tensor_sub(r1[:], x_ap, xh[:])
            nc.scalar.copy(xl[:], r1[:])
            nc.vector.tensor_sub(r2[:], r1[:], xl[:])
            nc.scalar.copy(xm[:], r2[:])
            return xh, xl, xm

        P = [dict() for _ in range(nb)]   # per-batch live tiles

        def emit_prep(b):
            p = P[b]
            kw = prep.tile([128, MCH, 3], dt.float32, tag="kw")
            nc.sync.dma_start(
                kw[:], AP(known_h, b * M * 3, [[3, 128], [3 * 128, MCH], [1, 3]])
            )
            k2 = prep.tile([128, MCH, 3], dt.float32, tag="k2")
            nc.vector.tensor_scalar_mul(k2[:], kw[:], 2.0)
            k2h, k2l, k2m = bf16_split3(k2[:], [128, MCH, 3])
            sq = prep.tile([128, MCH, 3], dt.float32, tag="ksq")
            nc.scalar.square(sq[:], kw[:])
            s_f = prep.tile([128, MCH], dt.float32, tag="ks")
            nc.vector.tensor_add(s_f[:], sq[:, :, 0], sq[:, :, 1])
            nc.vector.tensor_add(s_f[:], s_f[:], sq[:, :, 2])
            ns = prep.tile([128, MCH], dt.float32, tag="kns")
            nc.vector.tensor_scalar_mul(ns[:], s_f[:], -1.0)
            nsh, nsl, nsm = bf16_split3(ns[:], [128, MCH])

            kch = prep.tile([128, MCH, 24], dt.bfloat16, tag="kch")
            for (r0, src) in ((0, k2h), (3, k2l), (6, k2h), (9, k2l), (12, k2m), (15, k2h)):
                nc.scalar.copy(kch[:, :, r0:r0 + 3], src[:])
            nc.scalar.copy(kch[:, :, 18], nsh[:])
            nc.scalar.copy(kch[:, :, 19], nsl[:])
            nc.scalar.copy(kch[:, :, 20], nsm[:])
            rhs_all = prep.tile([KROWS, M], dt.bfloat16, tag="rhs_all")
            for t in range(MCH):
                pst = ps_tr.tile([32, 128], dt.bfloat16, tag="tr")
                nc.tensor.transpose(pst[:KROWS, :], kch[:, t, :KROWS], ident_b[:])
                eng = nc.vector.tensor_copy if (b == 0 and t % 2 == 0) else nc.scalar.copy
                eng(rhs_all[:, 128 * t:128 * t + 128], pst[:KROWS, :])

            uw = prep.tile([128, NCH, 3], dt.float32, tag="uw")
            nc.sync.dma_start(
                uw[:], AP(unknown_h, b * N * 3, [[3, 128], [3 * 128, NCH], [1, 3]])
            )
            uh, ul, um = bf16_split3(uw[:], [128, NCH, 3])
            uch = prep.tile([128, NCH, 24], dt.bfloat16, tag="uch")
            for (r0, src) in ((0, uh), (3, uh), (6, ul), (9, ul), (12, uh), (15, um)):
                nc.scalar.copy(uch[:, :, r0:r0 + 3], src[:])
            nc.vector.memset(uch[:, :, 18:21], 1.0)
            lhs_all = prep.tile([KROWS, N], dt.bfloat16, tag="lhs_all")
            for t in range(NCH):
                pst = ps_tr.tile([32, 128], dt.bfloat16, tag="tr")
                nc.tensor.transpose(pst[:KROWS, :], uch[:, t, :KROWS], ident_b[:])
                eng = nc.vector.tensor_copy if (b == 0 and t % 2 == 0) else nc.scalar.copy
                eng(lhs_all[:, 128 * t:128 * t + 128], pst[:KROWS, :])

            usqc = prep.tile([128, NCH, 3], dt.float32, tag="usqc")
            nc.scalar.square(usqc[:], uw[:])
            usq = prep.tile([128, NCH], dt.float32, tag="usq")
            nc.vector.tensor_add(usq[:], usqc[:, :, 0], usqc[:, :, 1])
            nc.vector.tensor_add(usq[:], usq[:], usqc[:, :, 2])
            p.update(lhs_all=lhs_all, rhs_all=rhs_all, usq=usq)

        def emit_staging(b):
            # ftsb[p, mt, :] = feats_t row (mt*128+p): dma_gather SBUF-source
            # layout (tokens_per_rank=128, free_dim_per_rank = C2*2 bytes)
            p = P[b]
            kf16 = ftp.tile([128, 4, M], dt.bfloat16, tag="kf16")
            for cj in range(4):
                for mh in range(2):
                    kf32 = ftp.tile([128, M // 2], dt.float32, tag="kf32")
                    nc.sync.dma_start(
                        kf32[:],
                        kf_h.ap()[b, 128 * cj:128 * cj + 128, 512 * mh:512 * mh + 512],
                    )
                    nc.scalar.copy(kf16[:, cj, 512 * mh:512 * mh + 512], kf32[:])
            ftsb = ftp.tile([128, MCH, C2], dt.bfloat16, tag="ftsb")
            for mt in range(MCH):
                for cj in range(4):
                    pst = ps_tr.tile([128, 128], dt.bfloat16, tag="tr")
                    nc.tensor.transpose(
                        pst[:], kf16[:, cj, 128 * mt:128 * mt + 128], ident_b[:]
                    )
                    nc.scalar.copy(ftsb[:, mt, 128 * cj:128 * cj + 128], pst[:])
            p.update(ftsb=ftsb)

        def emit_scan(b):
            p = P[b]
            vall = sel.tile([128, NCH, 8], dt.float32, tag="vall")
            miall = sel.tile([128, NCH, 8], dt.uint16, tag="miall")
            for t in range(NCH):
                psd = ps_d.tile([128, M], dt.float32, tag="psd")
                for hm in range(2):
                    nc.tensor.matmul(
                        psd[:, 512 * hm:512 * hm + 512],
                        p["lhs_all"][:, 128 * t:128 * t + 128],
                        p["rhs_all"][:, 512 * hm:512 * hm + 512],
                        start=True,
                        stop=True,
                    )
                nc.vector.max(out=vall[:, t, :], in_=psd[:])
                nc.vector.max_index(
                    out=miall[:, t, :], in_max=vall[:, t, :], in_values=psd[:]
                )
            p.update(vall=vall, miall=miall)

        def emit_post(b):
            # d2, neighbor indices (k-major fp16), weights (k-major bf16)
            p = P[b]
            vall, miall, usq = p["vall"], p["miall"], p["usq"]
            d23 = sel.tile([128, NCH, 3], dt.float32, tag="d23")
            nc.vector.tensor_sub(
                d23[:],
                usq[:].to_broadcast([128, NCH, 3]),
                _v(vall[:], [vall[:].ap[0], [8, NCH], [1, 3]]),
            )
            nc.vector.tensor_scalar(
                d23[:], d23[:], 0.0, EPS, op0=Alu.max, op1=Alu.add
            )
            j3f = sel.tile([128, NCH, 3], dt.float32, tag="j3f")
            nc.vector.tensor_copy(
                j3f[:], _v(miall[:], [miall[:].ap[0], [8, NCH], [1, 3]])
            )
            j3h = sel.tile([128, 3, NCH], dt.float16, tag="j3h")
            nc.vector.tensor_copy(
                j3h[:], _v(j3f[:], [j3f[:].ap[0], [1, 3], [3, NCH]])
            )
            r3 = wts.tile([128, NCH, 3], dt.float32, tag="r3")
            nc.vector.reciprocal(r3[:], d23[:])
            z = wts.tile([128, NCH], dt.float32, tag="z")
            nc.vector.tensor_reduce(z[:], r3[:], axis=mybir.AxisListType.X, op=Alu.add)
            iz = wts.tile([128, NCH], dt.float32, tag="iz")
            nc.vector.reciprocal(iz[:], z[:])
            w3f = wts.tile([128, NCH, 3], dt.float32, tag="w3f")
            nc.vector.tensor_mul(w3f[:], r3[:], iz[:].to_broadcast([128, NCH, 3]))
            w3b = wts.tile([128, 3, NCH], dt.bfloat16, tag="w3b")
            nc.scalar.copy(
                w3b[:], _v(w3f[:], [w3f[:].ap[0], [1, 3], [3, NCH]])
            )
            p.update(j3h=j3h, w3b=w3b)

        def emit_cons(b, h):
            # per-(half, k): wrow (r = 16p + t order), idxw, wb broadcast
            p = P[b]
            j3h, w3b = p["j3h"], p["w3b"]
            tsl = slice(HCH * h, HCH * h + HCH)
            idxws, wbs = [], []
            for k in range(3):
                wrow = wtmp.tile([1, HALF], dt.bfloat16, tag="wrow")
                nc.sync.dma_start(
                    _v(wrow[:], [wrow[:].ap[0], [16, 128], [1, HCH]]),
                    _v(w3b[:], [w3b[:].ap[0], [1, HCH]], off=NCH * k + HCH * h),
                )
                ps_ti = ps_tr.tile([16, 128], dt.float16, tag="tr")
                nc.tensor.transpose(ps_ti[:], j3h[:, k, tsl], ident_u[:])
                mit = wtmp.tile([16, 128], dt.float16, tag="mit")
                nc.scalar.copy(mit[:], ps_ti[:])
                ps_rr = ps_rp.tile([128, 128], dt.float32, tag="rr")
                nc.tensor.matmul(
                    ps_rr[:],
                    _v(r16[:], [r16[:].ap[0], [1, 128]]),
                    mit[:],
                    start=True,
                    stop=True,
                )
                idxw = wper.tile([128, 128], dt.int16, tag=f"idxw{k}")
                nc.vector.tensor_copy(idxw[:], ps_rr[:])

                wb = wper.tile([128, HALF], dt.bfloat16, tag=f"wb{k}")
                for nci in range(HALF // 512):
                    ps_wb = ps_mm.tile([128, 512], dt.float32, tag="mm")
                    nc.tensor.matmul(
                        ps_wb[:],
                        ones_b[:],
                        wrow[0:1, 512 * nci:512 * nci + 512],
                        start=True,
                        stop=True,
                    )
                    nc.scalar.copy(wb[:, 512 * nci:512 * nci + 512], ps_wb[:])
                idxws.append(idxw)
                wbs.append(wb)
            p[("cons", h)] = (idxws, wbs)

        def emit_uf(b, h):
            p = P[b]
            uf16 = mlpp.tile([128, 2, HALF], dt.bfloat16, tag="uf16")
            for cj in range(2):
                uf32 = ufp.tile([128, HALF], dt.float32, tag="uf32")
                nc.sync.dma_start(
                    uf32[:],
                    uf_h.ap()[b, 128 * cj:128 * cj + 128, HALF * h:HALF * h + HALF],
                )
                nc.scalar.copy(
                    uf16[:, cj, :],
                    _v(uf32[:], [uf32[:].ap[0], [1, 128], [128, HCH]]),
                )
            p[("uf16", h)] = uf16

        def emit_gather_wsum(b, h, q, q0=None, qn=QTR):
            # slice [q0, q0+qn) of half h, in half-local r units
            p = P[b]
            idxws, wbs = p[("cons", h)]
            if q0 is None:
                q0 = q * QTR
            interp = itp.tile([128, 4, qn], dt.bfloat16, tag="interp")
            gw2 = gwp.tile([128, 4, qn], dt.bfloat16, tag="gw")
            for k in range(3):
                g_t = gat.tile([128, 4, qn], dt.bfloat16, tag="g")
                nc.gpsimd.dma_gather(
                    g_t[:],
                    p["ftsb"][:],
                    idxws[k][:, q0 // 16:(q0 + qn) // 16],
                    qn,
                    qn,
                    C2,
                    transpose=True,
                    single_packet=False,
                    sbuf_tokens_per_rank=128,
                    sbuf_free_dim_per_rank=C2 * 2,
                )
                wbb = _v(wbs[k][:], [wbs[k][:].ap[0], [0, 4], [1, qn]], off=q0)
                if k == 0:
                    nc.vector.tensor_mul(interp[:], g_t[:], wbb)
                elif k == 1:
                    gw = gwp.tile([128, 4, qn], dt.bfloat16, tag="gw")
                    nc.vector.tensor_mul(gw[:], g_t[:], wbb)
                    nc.vector.tensor_add(interp[:], interp[:], gw[:])
                else:
                    # k=2 stays separate; MLP1 accumulates it in PSUM
                    nc.vector.tensor_mul(gw2[:], g_t[:], wbb)
            p[("wsum", h, q)] = (interp, gw2, q0, qn)

        def emit_mlp(b, h, q):
            p = P[b]
            interp, gw2, q0, qn = p.pop(("wsum", h, q))
            uf16 = p[("uf16", h)]
            h_t = mlpp.tile([128, 2, qn], dt.bfloat16, tag="h")
            for mj in range(2):
                for nci in range(qn // 512):
                    nsl_ = slice(512 * nci, 512 * nci + 512)
                    pm = ps_mm.tile([128, 512], dt.float32, tag="mm")
                    chunks = (
                        [(ci, interp[:, ci, nsl_]) for ci in range(4)]
                        + [(ci, gw2[:, ci, nsl_]) for ci in range(4)]
                        + [
                            (4 + ci, uf16[:, ci, q0 + 512 * nci:q0 + 512 * nci + 512])
                            for ci in range(2)
                        ]
                    )
                    for mi, (ci_w, rhs) in enumerate(chunks):
                        nc.tensor.matmul(
                            pm[:],
                            w0_sb[:, ci_w, 128 * mj:128 * mj + 128],
                            rhs,
                            start=(mi == 0),
                            stop=(mi == len(chunks) - 1),
                        )
                    nc.scalar.activation(h_t[:, mj, nsl_], pm[:], ACTF.Relu, bias=0.0)

            # MLP2 with r -> i un-permute into the full-half o_mj tiles
            p0 = q0 // HCH
            ppb = 512 // HCH
            for mj in range(2):
                o_t = p[("o", h, mj)]
                for nci in range(qn // 512):
                    nsl_ = slice(512 * nci, 512 * nci + 512)
                    pm = ps_mm.tile([128, 512], dt.float32, tag="mm")
                    for ci in range(2):
                        nc.tensor.matmul(
                            pm[:],
                            w1_sb[:, ci, 128 * mj:128 * mj + 128],
                            h_t[:, ci, nsl_],
                            start=(ci == 0),
                            stop=(ci == 1),
                        )
                    # pm col (p', t) -> o_t col 128*t + p0 + ppb*nci + p'
                    nc.scalar.activation(
                        _v(
                            o_t[:],
                            [o_t[:].ap[0], [1, ppb], [128, HCH]],
                            off=p0 + ppb * nci,
                        ),
                        pm[:],
                        ACTF.Relu,
                        bias=0.0,
                    )

        def emit_out(b, h):
            p = P[b]
            for mj in range(2):
                o_t = p.pop(("o", h, mj))
                nc.sync.dma_start(
                    out_h.ap()[b, 128 * mj:128 * mj + 128, HALF * h:HALF * h + HALF],
                    o_t[:],
                )

        def alloc_o(b, h):
            for mj in range(2):
                o_t = outp.tile([128, HALF], dt.float32, tag=f"o{mj}")
                P[b][("o", h, mj)] = o_t

        # ================= pipelined emission =================
        emit_prep(0)
        emit_staging(0)
        emit_scan(0)
        emit_post(0)
        for b in range(nb):
            last = b + 1 == nb
            emit_cons(b, 0)
            alloc_o(b, 0)
            emit_gather_wsum(b, 0, 0)
            emit_uf(b, 0)
            emit_gather_wsum(b, 0, 1)
            emit_cons(b, 1)
            emit_mlp(b, 0, 0)
            emit_gather_wsum(b, 1, 0)
            emit_uf(b, 1)
            alloc_o(b, 1)
            if not last:
                emit_prep(b + 1)
            emit_mlp(b, 0, 1)
            if not last:
                emit_out(b, 0)
                emit_gather_wsum(b, 1, 1)
                emit_staging(b + 1)
                emit_mlp(b, 1, 0)
                emit_scan(b + 1)
                emit_post(b + 1)
                emit_mlp(b, 1, 1)
                emit_out(b, 1)
            else:
                # drain the final half-quarters in 512-column strips
                emit_gather_wsum(b, 1, 1, q0=1024, qn=512)
                emit_out(b, 0)
                emit_mlp(b, 1, 0)
                emit_gather_wsum(b, 1, 2, q0=1536, qn=512)
                emit_mlp(b, 1, 1)
                emit_mlp(b, 1, 2)
                emit_out(b, 1)

    nc.compile()
    return nc


_NC_CACHE = {}


def _get_nc(nb=NB):
    if nb not in _NC_CACHE:
        _NC_CACHE[nb] = build_nc(nb)
    return _NC_CACHE[nb]


def kernel(**inputs):
    from concourse.bass_utils import run_bass_kernel_spmd

    nc = _get_nc(NB)
    per_core = B_FULL // N_CORES
    in_maps = []
    for c in range(N_CORES):
        sl = slice(per_core * c, per_core * (c + 1))
        in_maps.append(
            {
                "unknown": np.ascontiguousarray(np.asarray(inputs["unknown"][sl], dtype=np.float32)),
                "known": np.ascontiguousarray(np.asarray(inputs["known"][sl], dtype=np.float32)),
                "unknow_feats": np.ascontiguousarray(np.asarray(inputs["unknow_feats"][sl], dtype=np.float32)),
                "known_feats": np.ascontiguousarray(np.asarray(inputs["known_feats"][sl], dtype=np.float32)),
                "W0": np.asarray(inputs["W0"], dtype=np.float32),
                "W1": np.asarray(inputs["W1"], dtype=np.float32),
            }
        )
    res = run_bass_kernel_spmd(nc, in_maps, core_ids=list(range(N_CORES)))
    out = np.concatenate([res.results[c]["out"] for c in range(N_CORES)], axis=0)
    return out.astype(np.float32)
